# revision 1
# baseline (speedup 1.0000x reference)
"""Trainium2 Bass kernel for nn_CI3addFrom01 (segment_reduce).

Reference computation:
    out[b] = sum_m softmax(preweight)[m] * max_k min_j x[b, idx[m,k,j]]
with M = 40704 antichains over DIM = 32, enumerated systematically
(singletons; pairs x {((i,j)), ((i,),(j,))}; triples x 8 types).

Device formulation (no gathers — everything is matmul one-hot gathers,
rectangular broadcast access patterns, and fused multiply-reduce):
  Per core (M-axis sharded 8 ways), per 128-row batch tile:
    G_j = xT.T @ OH_j (j=0,1,2) one-hot matmuls over 838 "set" columns:
        [32 singletons | 62 pairs | 620 triples | 4x31 duplicated pairs X]
    SM = min(G0,G1,G2)   (min over each index set)
    MX = max(G0,G1,G2)   (max over each set; pairs+triples range only)
    xc regions, column-aligned with a host-packed weight row:
      R1  = SM[:, 0:714]                  1-group antichains (+singles core 0)
      R2  = MX[:, 0:682]                  ((i,),(j,)) and ((i,),(j,),(k,))
      R3a = max(SM_sing[a] , SM_pair[p])  grid [32 x 62]   singleton-vs-pair
      R3b = max(X[m,u], X[m,v])           grid [4 x 31 x 31] pair-vs-pair
    partial[b] += sum_c exp(pw_packed[c]) * xc[b, c]   (scalar_tensor_tensor)
  Invalid grid cells get pw = -1e30 (weight 0); R3b counts each antichain
  twice so its pw entries carry -ln(2). Host sums the 8 per-core partials
  and divides by Z = sum of per-core sum(exp(pw_packed)).
"""

import itertools
import math

import numpy as np

DIM = 32
B = 512
NCORES = 8
NPAIR_C = 62
NTRIP_C = 620
NM_C = 4
NTAB = 32 + NPAIR_C + NTRIP_C + NM_C * 31  # 838
NEG = -1e30

PAIRS = list(itertools.combinations(range(DIM), 2))
TRIPS = list(itertools.combinations(range(DIM), 3))
TRIPIDX = {t: i for i, t in enumerate(TRIPS)}

SPLIT_A = 15   # others[m][0:15]
SPLIT_B = 16   # others[m][15:31]
SEG = {}
_off = 0
for _name, _ln in [
    ("R1a", 32), ("R1b", NPAIR_C), ("R1c", NTRIP_C),
    ("R2b", NPAIR_C), ("R2c", NTRIP_C),
    ("R3a", 32 * NPAIR_C),
    ("R3r", NM_C * SPLIT_A * SPLIT_B),
    ("R3u", NM_C * SPLIT_A * SPLIT_A),
    ("R3v", NM_C * SPLIT_B * SPLIT_B),
]:
    SEG[_name] = (_off, _off + _ln)
    _off += _ln
LPACK = _off  # 6264
N_R1 = 32 + NPAIR_C + NTRIP_C  # 714
N_R2 = NPAIR_C + NTRIP_C       # 682
N_R3A = 32 * NPAIR_C           # 1984
N_R3B = SEG["R3v"][1] - SEG["R3r"][0]  # 2884


def _others(m):
    return [x for x in range(DIM) if x != m]


def _table_sets(core):
    sets = [(i, i, i) for i in range(DIM)]
    for p in range(NPAIR_C * core, NPAIR_C * (core + 1)):
        i, j = PAIRS[p]
        sets.append((i, j, j))
    for q in range(NTRIP_C * core, NTRIP_C * (core + 1)):
        sets.append(TRIPS[q])
    for m in range(NM_C * core, NM_C * (core + 1)):
        for x in _others(m):
            sets.append((min(m, x), max(m, x), max(m, x)))
    return sets


_HOST_CACHE = {}


def _onehots(core):
    if ("oh", core) in _HOST_CACHE:
        return _HOST_CACHE[("oh", core)]
    sets = np.asarray(_table_sets(core), dtype=np.int64)
    oh = np.zeros((3, DIM, NTAB), dtype=np.float32)
    for j in range(3):
        oh[j, sets[:, j], np.arange(NTAB)] = 1.0
    _HOST_CACHE[("oh", core)] = oh
    return oh


def _widx_r3a(core):
    if ("r3a", core) in _HOST_CACHE:
        return _HOST_CACHE[("r3a", core)]
    g = np.full((32, NPAIR_C), -1, dtype=np.int64)
    for a in range(32):
        for pl, p in enumerate(range(NPAIR_C * core, NPAIR_C * (core + 1))):
            b, c = PAIRS[p]
            if a == b or a == c:
                continue
            tri = tuple(sorted((a, b, c)))
            g[a, pl] = 1024 + 8 * TRIPIDX[tri] + 2 + tri.index(a)
    _HOST_CACHE[("r3a", core)] = g
    return g


def _r3b_widx_id(m, ou, ov):
    tri = tuple(sorted((m, ou, ov)))
    return 1024 + 8 * TRIPIDX[tri] + (7, 5, 6)[tri.index(m)]


def _widx_r3b(core):
    """Triangle-split grids: rect (A x B, full weight), triA (A x A, halved),
    triB (B x B, halved). -1 marks invalid (diagonal); -2 marks halved."""
    if ("r3b", core) in _HOST_CACHE:
        return _HOST_CACHE[("r3b", core)]
    rect = np.full((NM_C, SPLIT_A, SPLIT_B), -1, dtype=np.int64)
    tria = np.full((NM_C, SPLIT_A, SPLIT_A), -1, dtype=np.int64)
    trib = np.full((NM_C, SPLIT_B, SPLIT_B), -1, dtype=np.int64)
    halved_a = np.zeros_like(tria, dtype=bool)
    halved_b = np.zeros_like(trib, dtype=bool)
    for ml, m in enumerate(range(NM_C * core, NM_C * (core + 1))):
        ot = _others(m)
        for u in range(SPLIT_A):
            for v in range(SPLIT_B):
                rect[ml, u, v] = _r3b_widx_id(m, ot[u], ot[SPLIT_A + v])
        for u in range(SPLIT_A):
            for v in range(SPLIT_A):
                if u != v:
                    tria[ml, u, v] = _r3b_widx_id(m, ot[u], ot[v])
                    halved_a[ml, u, v] = True
        for u in range(SPLIT_B):
            for v in range(SPLIT_B):
                if u != v:
                    trib[ml, u, v] = _r3b_widx_id(m, ot[SPLIT_A + u],
                                                  ot[SPLIT_A + v])
                    halved_b[ml, u, v] = True
    res = (rect, tria, trib, halved_a, halved_b)
    _HOST_CACHE[("r3b", core)] = res
    return res


def _packed_pw(core, pw):
    pw = np.asarray(pw, dtype=np.float64).reshape(-1)
    out = np.full(LPACK, NEG, dtype=np.float64)
    if core == 0:
        out[SEG["R1a"][0]:SEG["R1a"][1]] = pw[0:32]
    p0, p1 = 32 + 2 * NPAIR_C * core, 32 + 2 * NPAIR_C * (core + 1)
    out[SEG["R1b"][0]:SEG["R1b"][1]] = pw[p0:p1:2]
    out[SEG["R2b"][0]:SEG["R2b"][1]] = pw[p0 + 1:p1 + 1:2]
    t0, t1 = 1024 + 8 * NTRIP_C * core, 1024 + 8 * NTRIP_C * (core + 1)
    out[SEG["R1c"][0]:SEG["R1c"][1]] = pw[t0:t1:8]
    out[SEG["R2c"][0]:SEG["R2c"][1]] = pw[t0 + 1:t1 + 1:8]
    rect, tria, trib, ha, hb = _widx_r3b(core)
    for name, grid, halved in [("R3a", _widx_r3a(core), None),
                               ("R3r", rect, None),
                               ("R3u", tria, ha),
                               ("R3v", trib, hb)]:
        flat = grid.reshape(-1)
        vals = np.full(flat.shape, NEG, dtype=np.float64)
        ok = flat >= 0
        vals[ok] = pw[flat[ok]]
        if halved is not None:
            vals[halved.reshape(-1)] -= math.log(2.0)
        s, e = SEG[name]
        out[s:e] = vals
    return out.astype(np.float32)


def _expected_idx():
    acs = [((i,),) for i in range(DIM)]
    for i, j in PAIRS:
        acs.append(((i, j),))
        acs.append(((i,), (j,)))
    for i, j, k in TRIPS:
        acs += [((i, j, k),), ((i,), (j,), (k,)), ((i,), (j, k)), ((j,), (i, k)),
                ((k,), (i, j)), ((i, j), (j, k)), ((i, k), (j, k)), ((i, j), (i, k))]
    idx = np.zeros((len(acs), 3, 3), dtype=np.int32)
    for m, ac in enumerate(acs):
        groups = [list(g) + [g[-1]] * (3 - len(g)) for g in ac]
        while len(groups) < 3:
            groups.append(groups[-1])
        idx[m] = np.array(groups, dtype=np.int32)
    return idx


_NC_CACHE = {}


_WP_BUFS = 4
_JP_BUFS = 2


def _build_nc(reps=1):
    import concourse.mybir as mybir
    from concourse import bacc
    from concourse.tile import TileContext

    f32 = mybir.dt.float32
    bf16 = mybir.dt.bfloat16
    Alu = mybir.AluOpType

    nc = bacc.Bacc(None, target_bir_lowering=False, debug=False)
    xT_d = nc.dram_tensor("xT", [DIM, B], bf16, kind="ExternalInput")
    oh_d = nc.dram_tensor("oh", [DIM, 3 * NTAB], bf16, kind="ExternalInput")
    pwp_d = nc.dram_tensor("pwp", [1, LPACK], bf16, kind="ExternalInput")
    out_d = nc.dram_tensor("out", [B, 1], f32, kind="ExternalOutput")
    zsum_d = nc.dram_tensor("zsum", [128, 1], f32, kind="ExternalOutput")

    with TileContext(nc) as tc:
        with (
            tc.tile_pool(name="const", bufs=1) as cp,
            tc.tile_pool(name="work", bufs=_WP_BUFS) as wp,
            tc.tile_pool(name="junkp", bufs=_JP_BUFS) as jp,
            tc.tile_pool(name="pe", bufs=2, space="PSUM") as pe_pool,
            tc.tile_pool(name="pg", bufs=1, space="PSUM") as pg_pool,
        ):
            oh_t = cp.tile([DIM, 3 * NTAB], bf16)
            xt_t = cp.tile([DIM, B], bf16)
            pwp_t = cp.tile([1, LPACK], bf16)
            ones1 = cp.tile([1, 128], bf16)
            E = cp.tile([128, LPACK], bf16)
            zparts = cp.tile([128, (LPACK + 511) // 512], f32)
            outb = cp.tile([128, 4], f32)
            nc.sync.dma_start(oh_t[:], oh_d[:])
            nc.sync.dma_start(xt_t[:], xT_d[:])
            nc.sync.dma_start(pwp_t[:], pwp_d[:])
            nc.vector.memset(ones1[:], 1.0)

            rep_blocks(nc, tc, mybir, f32, bf16, Alu, cp, wp, jp, pe_pool,
                       pg_pool, oh_t, xt_t, pwp_t, ones1, E, zparts, outb,
                       reps)

            # per-core softmax denominator: sum of the per-chunk exp accums
            zacc = cp.tile([128, 1], f32)
            nc.vector.tensor_reduce(zacc[:], zparts[:],
                                    axis=mybir.AxisListType.X, op=Alu.add)
            nc.sync.dma_start(zsum_d[:], zacc[:])

            for t in range(4):
                nc.sync.dma_start(out_d[t * 128:(t + 1) * 128, :],
                                  outb[:, t:t + 1])
    nc.finalize()
    return nc


def rep_blocks(nc, tc, mybir, f32, bf16, Alu, cp, wp, jp, pe_pool, pg_pool,
               oh_t, xt_t, pwp_t, ones1, E, zparts, outb, reps):
    for _rep in range(reps):
            # E = exp(pw_packed) broadcast to 128 partitions via ones-matmul;
            # per-chunk accum feeds the softmax denominator for free
            for ci, s in enumerate(range(0, LPACK, 512)):
                e = min(s + 512, LPACK)
                eb = pe_pool.tile([128, 512], f32, tag="eb")
                nc.tensor.matmul(eb[:, : e - s], ones1[:], pwp_t[:, s:e],
                                 start=True, stop=True)
                nc.scalar.activation(E[:, s:e], eb[:, : e - s],
                                     mybir.ActivationFunctionType.Exp,
                                     accum_out=zparts[:, ci:ci + 1])

            for t in range(4):
                g = []
                for j in range(3):
                    gj = pg_pool.tile([128, NTAB], f32, tag=f"g{j}")
                    for s in range(0, NTAB, 512):
                        e = min(s + 512, NTAB)
                        nc.tensor.matmul(
                            gj[:, s:e], xt_t[:, t * 128:(t + 1) * 128],
                            oh_t[:, j * NTAB + s: j * NTAB + e],
                            start=True, stop=True)
                    g.append(gj)

                # copy all three G tiles PSUM->SBUF on ACT so the DVE
                # min/max chain runs pure-SBUF bf16 (2x packed mode)
                c0 = wp.tile([128, NTAB], bf16, tag="c0")
                c1 = wp.tile([128, NTAB], bf16, tag="c1")
                c2 = wp.tile([128, NTAB], bf16, tag="c2")
                nc.scalar.copy(c0[:], g[0][:])
                nc.scalar.copy(c1[:], g[1][:])
                nc.scalar.copy(c2[:], g[2][:])
                t1 = wp.tile([128, NTAB], bf16, tag="t1")
                # xcall: [ SM(714) | MX(682) | R3a(1984) | R3b(2884) | X(124) ]
                # one contiguous tile so a single stt covers every region
                xcall = wp.tile([128, LPACK + NM_C * 31], bf16, tag="xcall")
                sm = xcall[:, 0:N_R1]
                mx = xcall[:, N_R1:N_R1 + N_R2]
                xs = xcall[:, LPACK:]
                nc.vector.tensor_tensor(t1[:], c0[:], c1[:], Alu.min)
                nc.vector.tensor_tensor(sm, t1[:, 0:N_R1], c2[:, 0:N_R1],
                                        Alu.min)
                nc.vector.tensor_tensor(xs, t1[:, N_R1:NTAB], c2[:, N_R1:NTAB],
                                        Alu.min)
                t2 = wp.tile([128, N_R2], bf16, tag="t2")
                nc.vector.tensor_tensor(t2[:], c0[:, 32:N_R1],
                                        c1[:, 32:N_R1], Alu.max)
                nc.vector.tensor_tensor(mx, t2[:], c2[:, 32:N_R1], Alu.max)

                r3a = xcall[:, SEG["R3a"][0]:SEG["R3a"][1]].rearrange(
                    "p (a q) -> p a q", a=32)
                nc.vector.tensor_tensor(
                    r3a,
                    sm[:, 0:32].unsqueeze(2).broadcast_to([128, 32, NPAIR_C]),
                    sm[:, 32:32 + NPAIR_C].unsqueeze(1)
                    .broadcast_to([128, 32, NPAIR_C]),
                    Alu.max)
                # R3b triangle split: rect(A x B) + triA + triB, one buffer
                r3b = xcall[:, SEG["R3r"][0]:SEG["R3v"][1]]
                xv = xs.rearrange("p (m t) -> p m t", m=NM_C)
                xa = xv[:, :, 0:SPLIT_A]
                xb = xv[:, :, SPLIT_A:31]
                nr = NM_C * SPLIT_A * SPLIT_B
                nu = NM_C * SPLIT_A * SPLIT_A
                nv = NM_C * SPLIT_B * SPLIT_B
                rect = r3b[:, 0:nr].rearrange("p (m u v) -> p m u v",
                                              m=NM_C, u=SPLIT_A)
                tria = r3b[:, nr:nr + nu].rearrange("p (m u v) -> p m u v",
                                                    m=NM_C, u=SPLIT_A)
                trib = r3b[:, nr + nu:nr + nu + nv].rearrange(
                    "p (m u v) -> p m u v", m=NM_C, u=SPLIT_B)
                nc.vector.tensor_tensor(
                    rect,
                    xa.unsqueeze(3).broadcast_to([128, NM_C, SPLIT_A, SPLIT_B]),
                    xb.unsqueeze(2).broadcast_to([128, NM_C, SPLIT_A, SPLIT_B]),
                    Alu.max)
                nc.vector.tensor_tensor(
                    tria,
                    xa.unsqueeze(3).broadcast_to([128, NM_C, SPLIT_A, SPLIT_A]),
                    xa.unsqueeze(2).broadcast_to([128, NM_C, SPLIT_A, SPLIT_A]),
                    Alu.max)
                nc.vector.tensor_tensor(
                    trib,
                    xb.unsqueeze(3).broadcast_to([128, NM_C, SPLIT_B, SPLIT_B]),
                    xb.unsqueeze(2).broadcast_to([128, NM_C, SPLIT_B, SPLIT_B]),
                    Alu.max)

                junk = jp.tile([128, LPACK], bf16, tag="junk")
                nc.vector.scalar_tensor_tensor(
                    junk[:], xcall[:, 0:LPACK], 1.0, E[:],
                    op0=Alu.mult, op1=Alu.mult, accum_out=outb[:, t:t + 1])


def make_in_maps(x, pw):
    import ml_dtypes

    bf = ml_dtypes.bfloat16
    xT = np.ascontiguousarray(np.asarray(x, np.float32).T.astype(bf))
    in_maps = []
    for core in range(NCORES):
        oh = _onehots(core)  # [3, 32, NTAB]
        in_maps.append({
            "xT": xT,
            "oh": np.ascontiguousarray(
                oh.transpose(1, 0, 2).reshape(DIM, 3 * NTAB).astype(bf)),
            "pwp": _packed_pw(core, pw).reshape(1, LPACK).astype(bf),
        })
    return in_maps


def kernel(x, preweight, idx):
    from concourse.bass_utils import run_bass_kernel_spmd

    x = np.ascontiguousarray(np.asarray(x, dtype=np.float32))
    pw = np.asarray(preweight, dtype=np.float32).reshape(-1)
    idx = np.asarray(idx)
    if not np.array_equal(idx, _expected_idx()):
        raise ValueError("idx does not match the expected antichain table")

    if "nc" not in _NC_CACHE:
        _NC_CACHE["nc"] = _build_nc()
    nc = _NC_CACHE["nc"]

    in_maps = make_in_maps(x, pw)
    res = run_bass_kernel_spmd(nc, in_maps, core_ids=list(range(NCORES)))
    total = np.zeros((B, 1), dtype=np.float64)
    z = 0.0
    for r in res.results:
        total += r["out"].astype(np.float64)
        z += float(r["zsum"][0, 0])
    return (total / z).astype(np.float32)


if __name__ == "__main__":
    rng = np.random.default_rng(11)
    x = rng.standard_normal((B, DIM)).astype(np.float32)
    pw = rng.standard_normal((1, 40704)).astype(np.float32)
    out = kernel(x, pw, _expected_idx())
    print("out", out.shape, out[:4, 0])



# revision 5
# speedup vs baseline: 1.4860x; 1.4860x over previous
"""Trainium2 Bass kernel for nn_CI3addFrom01 (segment_reduce).

Reference computation:
    out[b] = sum_m softmax(preweight)[m] * max_k min_j x[b, idx[m,k,j]]
with M = 40704 antichains over DIM = 32.

Device formulation (M-axis sharded 8 ways; per core, per 128-row batch
tile, everything is one-hot matmuls + rectangular broadcast min/max +
weighted accumulation against a host-precomputed exp(preweight) table):
    G_j = xT.T @ OH_j (j=0,1,2) over 838 "set" columns
    SM = min3(G)[:714]; MX = max3(G)[32:714]; X = min3(G)[714:838]
    xcall regions, column-aligned with a host-packed weight row E:
      R1 [0:714]     = SM              1-group antichains
      R2 [714:1396]  = MX              ((i,),(j,)) / ((i,),(j,),(k,))
      R3a [1396:3380] max(S_a, P_p)    [32 x 62] singleton-vs-pair grid
      R3b [3380:5784] max(X_u, X_v)    2-level triangle-split pair-vs-pair
    weighted sum: acc[b] += sum_c E_c * xcall[b, c], E = exp(pw_packed)
    (host computes exp and the softmax denominator; invalid grid slots
    get E = 0, double-counted triangle slots get E/2)
Work is balanced across DVE (bf16 2x chain + grids + mult), ACT (fused
PSUM->SBUF copy + accumulate-reduce), and GPSIMD/Pool (grids + fused stt).
Host sums the per-core partials and divides by Z = sum(exp(pw)).
"""

import itertools
import math

import numpy as np

DIM = 32
B = 512
NCORES = 8
NPAIR_C = 62
NTRIP_C = 620
NM_C = 4
NTAB = 32 + NPAIR_C + NTRIP_C + NM_C * 31  # 838
NEG = -1e30

PAIRS = list(itertools.combinations(range(DIM), 2))
TRIPS = list(itertools.combinations(range(DIM), 3))
TRIPIDX = {t: i for i, t in enumerate(TRIPS)}

# others[m] split: A = [0:15], B = [15:31]; A -> A1 [0:7], A2 [7:15];
# B -> B1 [0:8], B2 [8:16]
SPLIT_A = 15
SPLIT_B = 16
NA1, NA2 = 7, 8
NB1, NB2 = 8, 8

# R3b 2-level triangle split grids: (name, udim, vdim, uoff, voff, halved)
# offsets index into the per-m 31-col X slice
R3B_GRIDS = [
    ("rAB", SPLIT_A, SPLIT_B, 0, SPLIT_A, False),
    ("rA", NA1, NA2, 0, NA1, False),
    ("tA1", NA1, NA1, 0, 0, True),
    ("tA2", NA2, NA2, NA1, NA1, True),
    ("rB", NB1, NB2, SPLIT_A, SPLIT_A + NB1, False),
    ("tB1", NB1, NB1, SPLIT_A, SPLIT_A, True),
    ("tB2", NB2, NB2, SPLIT_A + NB1, SPLIT_A + NB1, True),
]

SEG = {}
_off = 0
for _name, _ln in (
    [("R1a", 32), ("R1b", NPAIR_C), ("R1c", NTRIP_C),
     ("R2b", NPAIR_C), ("R2c", NTRIP_C),
     ("R3a", 32 * NPAIR_C)]
    + [(n, NM_C * u * v) for n, u, v, _, _, _ in R3B_GRIDS]
):
    SEG[_name] = (_off, _off + _ln)
    _off += _ln
LPACK = _off  # 5784
N_R1 = 32 + NPAIR_C + NTRIP_C  # 714
N_R2 = NPAIR_C + NTRIP_C       # 682

# --- engine split tuning knobs ---
# wsum ranges: [0:W1] DVE stt; [W1:W2] DVE TT-mult + ACT accum-copy;
# [W2:LPACK] Pool stt (only if the toolchain can lower Pool elementwise —
# currently it cannot, so keep W2 = LPACK).  Grid ops assigned per name.
W1 = 0
W2 = LPACK
GRIDS_ON_POOL = set()


def _others(m):
    return [x for x in range(DIM) if x != m]


def _table_sets(core):
    sets = [(i, i, i) for i in range(DIM)]
    for p in range(NPAIR_C * core, NPAIR_C * (core + 1)):
        i, j = PAIRS[p]
        sets.append((i, j, j))
    for q in range(NTRIP_C * core, NTRIP_C * (core + 1)):
        sets.append(TRIPS[q])
    for m in range(NM_C * core, NM_C * (core + 1)):
        for x in _others(m):
            sets.append((min(m, x), max(m, x), max(m, x)))
    return sets


_HOST_CACHE = {}


def _onehots(core):
    if ("oh", core) in _HOST_CACHE:
        return _HOST_CACHE[("oh", core)]
    sets = np.asarray(_table_sets(core), dtype=np.int64)
    oh = np.zeros((3, DIM, NTAB), dtype=np.float32)
    for j in range(3):
        oh[j, sets[:, j], np.arange(NTAB)] = 1.0
    _HOST_CACHE[("oh", core)] = oh
    return oh


def _widx_r3a(core):
    if ("r3a", core) in _HOST_CACHE:
        return _HOST_CACHE[("r3a", core)]
    g = np.full((32, NPAIR_C), -1, dtype=np.int64)
    for a in range(32):
        for pl, p in enumerate(range(NPAIR_C * core, NPAIR_C * (core + 1))):
            b, c = PAIRS[p]
            if a == b or a == c:
                continue
            tri = tuple(sorted((a, b, c)))
            g[a, pl] = 1024 + 8 * TRIPIDX[tri] + 2 + tri.index(a)
    _HOST_CACHE[("r3a", core)] = g
    return g


def _r3b_widx_id(m, ou, ov):
    tri = tuple(sorted((m, ou, ov)))
    return 1024 + 8 * TRIPIDX[tri] + (7, 5, 6)[tri.index(m)]


def _widx_r3b(core):
    """Per 2-level grid: index table [NM_C, u, v] (-1 = invalid slot)."""
    if ("r3b", core) in _HOST_CACHE:
        return _HOST_CACHE[("r3b", core)]
    grids = {}
    for name, ud, vd, uo, vo, halved in R3B_GRIDS:
        g = np.full((NM_C, ud, vd), -1, dtype=np.int64)
        for ml, m in enumerate(range(NM_C * core, NM_C * (core + 1))):
            ot = _others(m)
            for u in range(ud):
                for v in range(vd):
                    gu, gv = ot[uo + u], ot[vo + v]
                    if gu != gv:
                        g[ml, u, v] = _r3b_widx_id(m, gu, gv)
        grids[name] = (g, halved)
    _HOST_CACHE[("r3b", core)] = grids
    return grids


def _packed_pwe(core, pw):
    """exp of the packed preweight row: the weighted-sum table E."""
    pw = np.asarray(pw, dtype=np.float64).reshape(-1)
    out = np.full(LPACK, NEG, dtype=np.float64)
    if core == 0:
        out[SEG["R1a"][0]:SEG["R1a"][1]] = pw[0:32]
    p0, p1 = 32 + 2 * NPAIR_C * core, 32 + 2 * NPAIR_C * (core + 1)
    out[SEG["R1b"][0]:SEG["R1b"][1]] = pw[p0:p1:2]
    out[SEG["R2b"][0]:SEG["R2b"][1]] = pw[p0 + 1:p1 + 1:2]
    t0, t1 = 1024 + 8 * NTRIP_C * core, 1024 + 8 * NTRIP_C * (core + 1)
    out[SEG["R1c"][0]:SEG["R1c"][1]] = pw[t0:t1:8]
    out[SEG["R2c"][0]:SEG["R2c"][1]] = pw[t0 + 1:t1 + 1:8]
    items = [("R3a", _widx_r3a(core), False)]
    r3b = _widx_r3b(core)
    for name, _, _, _, _, _ in R3B_GRIDS:
        g, halved = r3b[name]
        items.append((name, g, halved))
    for name, grid, halved in items:
        flat = grid.reshape(-1)
        vals = np.full(flat.shape, NEG, dtype=np.float64)
        ok = flat >= 0
        vals[ok] = pw[flat[ok]]
        if halved:
            vals[ok] -= math.log(2.0)
        s, e = SEG[name]
        out[s:e] = vals
    return np.exp(out)


def _expected_idx():
    acs = [((i,),) for i in range(DIM)]
    for i, j in PAIRS:
        acs.append(((i, j),))
        acs.append(((i,), (j,)))
    for i, j, k in TRIPS:
        acs += [((i, j, k),), ((i,), (j,), (k,)), ((i,), (j, k)), ((j,), (i, k)),
                ((k,), (i, j)), ((i, j), (j, k)), ((i, k), (j, k)), ((i, j), (i, k))]
    idx = np.zeros((len(acs), 3, 3), dtype=np.int32)
    for m, ac in enumerate(acs):
        groups = [list(g) + [g[-1]] * (3 - len(g)) for g in ac]
        while len(groups) < 3:
            groups.append(groups[-1])
        idx[m] = np.array(groups, dtype=np.int32)
    return idx


_NC_CACHE = {}

_WP_BUFS = 3
_JP_BUFS = 2


def _build_nc(reps=1):
    import concourse.mybir as mybir
    from concourse import bacc
    from concourse.tile import TileContext

    f32 = mybir.dt.float32
    bf16 = mybir.dt.bfloat16

    nc = bacc.Bacc(None, target_bir_lowering=False, debug=False)
    xT_d = nc.dram_tensor("xT", [DIM, B], bf16, kind="ExternalInput")
    oh_d = nc.dram_tensor("oh", [DIM, 3 * NTAB], bf16, kind="ExternalInput")
    pwe_d = nc.dram_tensor("pwe", [1, LPACK], bf16, kind="ExternalInput")
    outd_d = nc.dram_tensor("outd", [B, 1], f32, kind="ExternalOutput")
    outa_d = nc.dram_tensor("outa", [B, 1], f32, kind="ExternalOutput")
    outp_d = nc.dram_tensor("outp", [B, 1], f32, kind="ExternalOutput")

    with TileContext(nc) as tc:
        with (
            tc.tile_pool(name="const", bufs=1) as cp,
            tc.tile_pool(name="ep", bufs=2) as ep,
            tc.tile_pool(name="work", bufs=_WP_BUFS) as wp,
            tc.tile_pool(name="junkp", bufs=_JP_BUFS) as jp,
            tc.tile_pool(name="pg", bufs=1, space="PSUM") as pg_pool,
        ):
            oh_t = cp.tile([DIM, 3 * NTAB], bf16)
            xt_t = cp.tile([DIM, B], bf16)
            outd_b = cp.tile([128, 4], f32)
            outa_b = cp.tile([128, 4], f32)
            outp_b = cp.tile([128, 4], f32)
            nc.sync.dma_start(oh_t[:], oh_d[:])
            nc.sync.dma_start(xt_t[:], xT_d[:])

            rep_blocks(nc, tc, mybir, f32, bf16, cp, ep, wp, jp, pg_pool,
                       oh_t, xt_t, pwe_d, outd_b, outa_b, outp_b, reps)

            for t in range(4):
                nc.sync.dma_start(outd_d[t * 128:(t + 1) * 128, :],
                                  outd_b[:, t:t + 1])
                nc.sync.dma_start(outa_d[t * 128:(t + 1) * 128, :],
                                  outa_b[:, t:t + 1])
                nc.sync.dma_start(outp_d[t * 128:(t + 1) * 128, :],
                                  outp_b[:, t:t + 1])
    nc.finalize()
    return nc


def rep_blocks(nc, tc, mybir, f32, bf16, cp, ep, wp, jp, pg_pool,
               oh_t, xt_t, pwe_d, outd_b, outa_b, outp_b, reps):
    Alu = mybir.AluOpType
    Act = mybir.ActivationFunctionType
    for _rep in range(reps):
        # E = exp(pw_packed) broadcast to 128 partitions via DMA
        # (split over two queues; double-buffered across reps)
        E = ep.tile([128, LPACK], bf16, tag="E")
        h = (LPACK // 2) & ~127
        nc.sync.dma_start(E[:, 0:h],
                          pwe_d[0:1, 0:h].broadcast_to([128, h]))
        nc.scalar.dma_start(E[:, h:LPACK],
                            pwe_d[0:1, h:LPACK].broadcast_to([128, LPACK - h]))

        for t in range(4):
            # G: one PSUM tile, j-blocks at col j*1024 so every 512-col
            # matmul chunk is PSUM-bank aligned
            G = pg_pool.tile([128, 3 * 1024], f32, tag="G")
            for j in range(3):
                for s in range(0, NTAB, 512):
                    e = min(s + 512, NTAB)
                    nc.tensor.matmul(
                        G[:, j * 1024 + s: j * 1024 + e],
                        xt_t[:, t * 128:(t + 1) * 128],
                        oh_t[:, j * NTAB + s: j * NTAB + e],
                        start=True, stop=True)

            # single fused PSUM->SBUF bf16 copy of all three j-blocks
            c012 = wp.tile([128, 3 * NTAB], bf16, tag="c012")
            nc.scalar.copy(
                c012[:].rearrange("p (j c) -> p j c", j=3),
                G[:].rearrange("p (j c) -> p j c", j=3, c=1024)[:, :, 0:NTAB])
            c0 = c012[:, 0:NTAB]
            c1 = c012[:, NTAB:2 * NTAB]
            c2 = c012[:, 2 * NTAB:3 * NTAB]

            xcall = wp.tile([128, LPACK], bf16, tag="xcall")
            sm = xcall[:, 0:N_R1]
            mx = xcall[:, N_R1:N_R1 + N_R2]

            # X = min3 over the dup-pair tail [714:838] (R3b grid operands)
            xs1 = wp.tile([128, NM_C * 31], bf16, tag="xs1")
            xs = wp.tile([128, NM_C * 31], bf16, tag="xs")
            nc.vector.tensor_tensor(xs1[:], c0[:, N_R1:NTAB],
                                    c1[:, N_R1:NTAB], Alu.min)
            nc.vector.tensor_tensor(xs[:], xs1[:], c2[:, N_R1:NTAB], Alu.min)

            # R1 = min3, R2 = max3 (bf16 2x chain)
            t1 = wp.tile([128, N_R1], bf16, tag="t1")
            nc.vector.tensor_tensor(t1[:], c0[:, 0:N_R1], c1[:, 0:N_R1],
                                    Alu.min)
            nc.vector.tensor_tensor(sm, t1[:], c2[:, 0:N_R1], Alu.min)
            t2 = wp.tile([128, N_R2], bf16, tag="t2")
            nc.vector.tensor_tensor(t2[:], c0[:, 32:N_R1], c1[:, 32:N_R1],
                                    Alu.max)
            nc.vector.tensor_tensor(mx, t2[:], c2[:, 32:N_R1], Alu.max)

            # R3a grid [32 x 62]: max(S_a, P_p)
            r3a = xcall[:, SEG["R3a"][0]:SEG["R3a"][1]].rearrange(
                "p (a q) -> p a q", a=32)
            nc.vector.tensor_tensor(
                r3a,
                sm[:, 0:32].unsqueeze(2).broadcast_to([128, 32, NPAIR_C]),
                sm[:, 32:32 + NPAIR_C].unsqueeze(1)
                .broadcast_to([128, 32, NPAIR_C]),
                Alu.max)

            # R3b 2-level triangle-split grids over X
            xv = xs[:].rearrange("p (m t) -> p m t", m=NM_C)
            for name, ud, vd, uo, vo, _ in R3B_GRIDS:
                s, e = SEG[name]
                dst = xcall[:, s:e].rearrange("p (m u v) -> p m u v",
                                              m=NM_C, u=ud)
                ueng = nc.gpsimd if name in GRIDS_ON_POOL else nc.vector
                ueng.tensor_tensor(
                    dst,
                    xv[:, :, uo:uo + ud].unsqueeze(3)
                    .broadcast_to([128, NM_C, ud, vd]),
                    xv[:, :, vo:vo + vd].unsqueeze(2)
                    .broadcast_to([128, NM_C, ud, vd]),
                    Alu.max)

            # weighted sum: 3-way engine split
            junk = jp.tile([128, LPACK], bf16, tag="junk")
            junk2 = jp.tile([128, W2 - W1], bf16, tag="junk2")
            if W1 > 0:
                nc.vector.scalar_tensor_tensor(
                    junk[:, 0:W1], xcall[:, 0:W1], 1.0, E[:, 0:W1],
                    op0=Alu.mult, op1=Alu.mult, accum_out=outd_b[:, t:t + 1])
            else:
                nc.vector.memset(outd_b[:, t:t + 1], 0.0)
            if W2 > W1:
                nc.vector.tensor_tensor(junk[:, W1:W2], xcall[:, W1:W2],
                                        E[:, W1:W2], Alu.mult)
                nc.scalar.activation(junk2[:], junk[:, W1:W2], Act.Copy,
                                     accum_out=outa_b[:, t:t + 1])
            else:
                nc.vector.memset(outa_b[:, t:t + 1], 0.0)
            if LPACK > W2:
                nc.gpsimd.scalar_tensor_tensor(
                    junk[:, W2:LPACK], xcall[:, W2:LPACK], 1.0, E[:, W2:LPACK],
                    op0=Alu.mult, op1=Alu.mult, accum_out=outp_b[:, t:t + 1])
            else:
                nc.vector.memset(outp_b[:, t:t + 1], 0.0)


def make_in_maps(x, pw):
    import ml_dtypes

    bf = ml_dtypes.bfloat16
    xT = np.ascontiguousarray(np.asarray(x, np.float32).T.astype(bf))
    in_maps = []
    for core in range(NCORES):
        oh = _onehots(core)  # [3, 32, NTAB]
        in_maps.append({
            "xT": xT,
            "oh": np.ascontiguousarray(
                oh.transpose(1, 0, 2).reshape(DIM, 3 * NTAB).astype(bf)),
            "pwe": _packed_pwe(core, pw).reshape(1, LPACK).astype(bf),
        })
    return in_maps


def kernel(x, preweight, idx):
    from concourse.bass_utils import run_bass_kernel_spmd

    x = np.ascontiguousarray(np.asarray(x, dtype=np.float32))
    pw = np.asarray(preweight, dtype=np.float32).reshape(-1)
    idx = np.asarray(idx)
    if not np.array_equal(idx, _expected_idx()):
        raise ValueError("idx does not match the expected antichain table")

    if "nc" not in _NC_CACHE:
        _NC_CACHE["nc"] = _build_nc()
    nc = _NC_CACHE["nc"]

    in_maps = make_in_maps(x, pw)
    res = run_bass_kernel_spmd(nc, in_maps, core_ids=list(range(NCORES)))
    total = np.zeros((B, 1), dtype=np.float64)
    for r in res.results:
        total += r["outd"].astype(np.float64)
        total += r["outa"].astype(np.float64)
        total += r["outp"].astype(np.float64)
    z = float(np.sum(np.exp(pw.astype(np.float64))))
    return (total / z).astype(np.float32)


if __name__ == "__main__":
    rng = np.random.default_rng(11)
    x = rng.standard_normal((B, DIM)).astype(np.float32)
    pw = rng.standard_normal((1, 40704)).astype(np.float32)
    out = kernel(x, pw, _expected_idx())
    print("out", out.shape, out[:4, 0])
